# revision 1
# baseline (speedup 1.0000x reference)
"""NT-Xent contrastive loss on 8 Trainium2 NeuronCores.

Strategy (row-sharded sim matrix, no collectives):
  - Every core receives the FULL proj_1/proj_2 plus its own 1024-row slice
    (rows_a) of concat(z_i, z_j) and the matching partner slice (rows_b).
  - Each core normalizes all 8192 rows, builds the transposed bf16
    embedding matrix zT [128d x 8192] as 16 [128,512] chunk tiles,
    computes its 1024x8192 block of exp(2*sim) row-sums via PE matmuls +
    ACT exp-with-accumulate, and emits per-row partials ln(denom) - 2*pos.
  - Host sums the 8 partial outputs -> scalar loss.

The ACT (scalar) engine carries the irreducible 8.4M exp evaluations per
core (~62us busy), so everything else stays off it: norms on DVE,
transposes on PE (first half, before the main loop owns PSUM) and on the
DMA xbar (second half, overlapped under the exp stream). 1/sqrt(n2) is
exp(-0.5*ln(n2)) computed in two batched Ln+Exp pairs -- Ln and Exp live
in different ACT table sets in this toolchain (a set switch costs ~1.3us),
so per-tile inv ops would thrash the table loader; batching caps it at
~5 switches, with data deps (not scheduler hints) enforcing the order.
"""

import sys

sys.path.insert(0, "/opt/trn_rl_repo")

import numpy as np

BATCH = 4096
DIM = 128
NCORES = 8
RPC = 2 * BATCH // NCORES  # 1024 rows per core
E2 = float(np.exp(2.0))  # exp(sim_gg / T) for the masked diagonal, sim_gg == 1

_CACHE = {}


def _build_nc():
    import concourse.bacc as bacc
    import concourse.bass as bass
    import concourse.mybir as mybir
    import concourse.tile as tile

    fp32 = mybir.dt.float32
    bf16 = mybir.dt.bfloat16
    AF = mybir.ActivationFunctionType
    ALU = mybir.AluOpType
    AX = mybir.AxisListType

    nc = bacc.Bacc("TRN2", target_bir_lowering=False, debug=False, num_devices=NCORES)
    p1 = nc.declare_dram_parameter("proj_1", [BATCH, DIM], fp32, isOutput=False)
    p2 = nc.declare_dram_parameter("proj_2", [BATCH, DIM], fp32, isOutput=False)
    ra = nc.declare_dram_parameter("rows_a", [RPC, DIM], fp32, isOutput=False)
    rb = nc.declare_dram_parameter("rows_b", [RPC, DIM], fp32, isOutput=False)
    out = nc.declare_dram_parameter("partial", [128, 8], fp32, isOutput=True)

    with tile.TileContext(nc) as tc:
        with (
            tc.tile_pool(name="big", bufs=1) as big,
            tc.tile_pool(name="jk", bufs=3) as jk,
        ):
            # DRAM views: contiguous per partition (partition p <- 8 rows/tile)
            src1 = p1[:].rearrange("(p a) d -> p (a d)", p=128)
            src2 = p2[:].rearrange("(p a) d -> p (a d)", p=128)
            srca = ra[:].rearrange("(p a) d -> p (a d)", p=128)
            srcb = rb[:].rearrange("(p a) d -> p (a d)", p=128)

            # source tiles [128, 1024]; load order: group1 first, xb last
            g1_names = ["xa", "x1_0", "x1_1", "x1_2", "x1_3"]
            g2_names = ["x2_0", "x2_1", "x2_2", "x2_3", "xb"]
            srcs = {"xa": srca, "xb": srcb}
            for m in range(4):
                srcs[f"x1_{m}"] = src1[:, 1024 * m : 1024 * (m + 1)]
                srcs[f"x2_{m}"] = src2[:, 1024 * m : 1024 * (m + 1)]
            xt = {}
            for name in g1_names + g2_names:
                t = big.tile([128, 1024], fp32, tag=name)
                nc.sync.dma_start(t[:], srcs[name])
                xt[name] = t

            # zT chunk tiles [128, 512] bf16 + zaT chunks
            zTc = []
            for c in range(16):
                zTc_t = big.tile([128, 512], bf16, tag=f"zT{c}")
                zTc.append(zTc_t)
            zaTc = []
            for c in range(2):
                zaTc_t = big.tile([128, 512], bf16, tag=f"zaT{c}")
                zaTc.append(zaTc_t)

            # identity for PE transposes (built on otherwise-idle GPSIMD)
            ident = big.tile([128, 128], bf16, tag="ident")
            ones1 = big.tile([128, 128], bf16, tag="ones1")
            nc.gpsimd.memset(ones1[:], 1.0)
            nc.gpsimd.affine_select(
                ident[:], ones1[:], [[1, 128]], ALU.is_equal, 0.0,
                base=0, channel_multiplier=-1,
            )

            def norms_into(n2g, names):
                for i, name in enumerate(names):
                    sq = jk.tile([128, 1024], fp32, tag="sq")
                    nc.gpsimd.tensor_mul(sq[:], xt[name][:], xt[name][:])
                    nc.vector.tensor_reduce(
                        n2g[:, 8 * i : 8 * (i + 1)],
                        sq[:].rearrange("p (a d) -> p a d", d=128),
                        axis=AX.X, op=ALU.add,
                    )

            def inv_of(n2g, label, w=40):
                lng = big.tile([128, w], fp32, tag=f"ln_{label}")
                nc.scalar.activation(lng[:], n2g[:], AF.Ln)
                invg = big.tile([128, w], fp32, tag=f"inv_{label}")
                nc.scalar.activation(invg[:], lng[:], AF.Exp, scale=-0.5)
                return invg

            def scale_tile(name, invg, i):
                z = big.tile([128, 1024], bf16, tag=f"z_{name}")
                for j in range(8):
                    nc.vector.tensor_scalar(
                        z[:, 128 * j : 128 * (j + 1)],
                        xt[name][:, 128 * j : 128 * (j + 1)],
                        invg[:, 8 * i + j : 8 * i + j + 1], None, op0=ALU.mult,
                    )
                return z

            def pe_transpose_group(z, b0, dst):
                pt = tp.tile([128, 512], fp32, tag="pt")
                for q in range(4):
                    nc.tensor.matmul(
                        pt[:, 128 * q : 128 * (q + 1)],
                        z[:, 128 * (b0 + q) : 128 * (b0 + q + 1)],
                        ident[:], start=True, stop=True,
                    )
                nc.vector.tensor_copy(dst[:], pt[:])

            # ---- group 1: xa + x1 -> inv -> scales -> PE transposes
            # (two inv batches so early zT chunks unblock the PE FIFO sooner)
            n2g1a = big.tile([128, 24], fp32, tag="n2g1a")
            norms_into(n2g1a, g1_names[:3])
            invg1 = inv_of(n2g1a, "g1a", 24)
            n2g1b = big.tile([128, 16], fp32, tag="n2g1b")
            norms_into(n2g1b, g1_names[3:])
            invg1b = inv_of(n2g1b, "g1b", 16)
            with tc.tile_pool(name="tp", bufs=3, space=bass.MemorySpace.PSUM) as tp:
                za = scale_tile("xa", invg1, 0)
                pe_transpose_group(za, 0, zaTc[0])
                pe_transpose_group(za, 4, zaTc[1])
                for m in range(2):
                    z = scale_tile(f"x1_{m}", invg1, m + 1)
                    pe_transpose_group(z, 0, zTc[2 * m])
                    pe_transpose_group(z, 4, zTc[2 * m + 1])
            # late x1 tiles go through the DMA xbar like x2 (keeps the PSUM
            # transpose pool short-lived so the main loop's pool starts early)
            for m in (2, 3):
                z = scale_tile(f"x1_{m}", invg1b, m - 2)
                for b in range(8):
                    c = 2 * m + b // 4
                    nc.sync.dma_start_transpose(
                        zTc[c][:, 128 * (b % 4) : 128 * (b % 4 + 1)],
                        z[:, 128 * b : 128 * (b + 1)],
                    )

            # ---- group 2: x2 + xb -> inv -> scales -> DMA-xbar transposes
            n2g2 = big.tile([128, 40], fp32, tag="n2g2")
            norms_into(n2g2, g2_names)
            invg2 = inv_of(n2g2, "g2")
            for m in range(4):
                z = scale_tile(f"x2_{m}", invg2, m)
                for b in range(8):
                    c = 8 + 2 * m + b // 4
                    nc.sync.dma_start_transpose(
                        zTc[c][:, 128 * (b % 4) : 128 * (b % 4 + 1)],
                        z[:, 128 * b : 128 * (b + 1)],
                    )

            # positives: D[p,j] = rows_a[8p+j] . rows_b[8p+j] (raw fp32 dots)
            pd = jk.tile([128, 1024], fp32, tag="sq")
            nc.vector.tensor_mul(pd[:], xt["xa"][:], xt["xb"][:])
            D = big.tile([128, 8], fp32, tag="D")
            nc.vector.tensor_reduce(
                D[:], pd[:].rearrange("p (a d) -> p a d", d=128),
                axis=AX.X, op=ALU.add,
            )

            # ---- main loop: phase A (h=0,1) uses zT chunks 0..7 (PE),
            # phase B (h=2,3) uses chunks 8..15 (DMA xbar, arriving meanwhile)
            RS = big.tile([128, 32], fp32, tag="RS")
            with tc.tile_pool(name="psum", bufs=2, space=bass.MemorySpace.PSUM) as pp:
                for h in (0, 1, 2, 3):
                    for j in range(8):
                        lhsT = zaTc[j // 4][:, 128 * (j % 4) : 128 * (j % 4 + 1)]
                        ps = pp.tile([128, 2048], fp32, tag="ps")
                        for q in range(4):
                            c = 4 * h + q
                            nc.tensor.matmul(
                                ps[:, 512 * q : 512 * (q + 1)], lhsT, zTc[c][:],
                                start=True, stop=True,
                            )
                        je = jk.tile([128, 2048], bf16, tag="je")
                        nc.scalar.activation(
                            je[:], ps[:], AF.Exp, scale=2.0,
                            accum_out=RS[:, 4 * j + h : 4 * j + h + 1],
                        )

            # ---- tail: partial[p,j] = ln(rowsum - e^2) - 2 * pos
            rs8 = big.tile([128, 8], fp32, tag="rs8")
            nc.vector.tensor_reduce(
                rs8[:], RS[:].rearrange("p (a c) -> p a c", c=4),
                axis=AX.X, op=ALU.add,
            )
            lnv = big.tile([128, 8], fp32, tag="lnv")
            nege2 = big.tile([128, 1], fp32, tag="nege2")
            nc.gpsimd.memset(nege2[:], -E2)
            nc.scalar.activation(lnv[:], rs8[:], AF.Ln, bias=nege2[:])
            t1 = big.tile([128, 8], fp32, tag="t1")
            nc.vector.tensor_mul(t1[:], D[:], invg1[:, 0:8])
            pos2 = big.tile([128, 8], fp32, tag="pos2")
            nc.vector.tensor_mul(pos2[:], t1[:], invg2[:, 32:40])
            p2t = big.tile([128, 8], fp32, tag="p2t")
            nc.vector.tensor_scalar(p2t[:], pos2[:], 2.0, None, op0=ALU.mult)
            res = big.tile([128, 8], fp32, tag="res")
            nc.vector.tensor_sub(res[:], lnv[:], p2t[:])
            nc.sync.dma_start(out[:], res[:])

    nc.compile()
    return nc


def _get_nc():
    if "nc" not in _CACHE:
        _CACHE["nc"] = _build_nc()
    return _CACHE["nc"]


def _in_maps(proj_1, proj_2):
    p1 = np.ascontiguousarray(np.asarray(proj_1, dtype=np.float32))
    p2 = np.ascontiguousarray(np.asarray(proj_2, dtype=np.float32))
    X = np.concatenate([p1, p2], axis=0)
    maps = []
    for k in range(NCORES):
        g0 = RPC * k
        pg = g0 + BATCH if g0 < BATCH else g0 - BATCH
        maps.append(
            {
                "proj_1": p1,
                "proj_2": p2,
                "rows_a": np.ascontiguousarray(X[g0 : g0 + RPC]),
                "rows_b": np.ascontiguousarray(X[pg : pg + RPC]),
            }
        )
    return maps


def _run(proj_1, proj_2, trace=False):
    from concourse.bass_utils import run_bass_kernel_spmd

    nc = _get_nc()
    res = run_bass_kernel_spmd(
        nc, _in_maps(proj_1, proj_2), list(range(NCORES)), trace=trace
    )
    tot = 0.0
    for k in range(NCORES):
        tot += float(res.results[k]["partial"].sum(dtype=np.float64))
    loss = np.float32(tot / (2 * BATCH))
    return loss, res


def kernel(proj_1, proj_2):
    loss, _ = _run(proj_1, proj_2, trace=False)
    return loss



# revision 3
# speedup vs baseline: 1.0222x; 1.0222x over previous
"""NT-Xent contrastive loss on 8 Trainium2 NeuronCores — v2.

Exploits sim-matrix symmetry to HALVE the exp work vs v1 (the ACT
engine's exp stream is the critical path): the 8192x8192 sim matrix is
cut into a 16x16 grid of 512-row chunks; each unordered chunk pair is
computed ONCE. Core k owns block-rows k and k+8 (circulant): row k
sweeps columns k..k+8, row k+8 sweeps k+8..k+15 (mod 16) — 17 blocks
per core (9+8), exactly 1/8 of the 136 upper-triangle blocks.
Row-sums of each exp block come free via ACT accumulate; the mirrored
row-sums (transposed blocks) come from column-sums computed on the PE
with ones-column lhsT matmuls into a persistent [16,512] PSUM
accumulator. Host assembles rowsums, subtracts the diagonal e^2, and
does the final ln/sum in float64 (off the HW critical path).

Inputs are chunk-ROTATED per core on the host (local chunk c = global
chunk (k+c) mod 16), so the device program is identical across cores.

Head avoids ACT-table thrash entirely: 1/sqrt(n2) is a quake-style
bit-trick rsqrt on DVE (shift/xor on bitcast int32 + 2 Newton steps),
so ACT only ever runs Square/Copy/Exp — all present in the exp table
set (a dummy exp up front pins it; its single table load lands in the
idle boot window). Squares for chunks 0..8 run on ACT (idle until the
exp stream starts), 9..15 on GpSimd; scales split GpSimd/DVE;
PSUM->SBUF transpose copies go ACT (first 3, pre-stream) / DVE (rest).
Emission order is hand-scheduled so no in-order engine queue blocks
the ACT exp stream on a late-chunk dependency.
"""

import sys

sys.path.insert(0, "/opt/trn_rl_repo")

import numpy as np

BATCH = 4096
DIM = 128
NCORES = 8
NCH = 16  # 512-row chunks of concat(z1, z2)
CHR = 512
E2 = float(np.exp(2.0))
MAGIC = 0x5F3759DF

# (lhs chunk, rhs chunks) per PSUM group; RS col = 4*gi + sub
GROUPS = [
    (0, [0, 1, 2]),
    (0, [3, 4, 5]),
    (0, [6, 7, 8]),
    (8, [8, 9, 10]),
    (8, [11, 12, 13]),
    (8, [14, 15]),
]

_CACHE = {}


def _build_nc():
    import concourse.bacc as bacc
    import concourse.bass as bass
    import concourse.mybir as mybir
    import concourse.tile as tile

    fp32 = mybir.dt.float32
    bf16 = mybir.dt.bfloat16
    i32 = mybir.dt.int32
    AF = mybir.ActivationFunctionType
    ALU = mybir.AluOpType
    AX = mybir.AxisListType

    nc = bacc.Bacc("TRN2", target_bir_lowering=False, debug=False, num_devices=NCORES)
    xr = nc.declare_dram_parameter("xrot", [2 * BATCH, DIM], fp32, isOutput=False)
    rowp_out = nc.declare_dram_parameter("rowp", [128, 12], fp32, isOutput=True)
    colp_out = nc.declare_dram_parameter("colp", [16, 512], fp32, isOutput=True)

    with tile.TileContext(nc) as tc:
        with (
            tc.tile_pool(name="big", bufs=1) as big,
            tc.tile_pool(name="sqp", bufs=9) as sqp,
            tc.tile_pool(name="jep", bufs=9) as jep,
        ):
            # ---- input loads: 16 chunk tiles [128, 512] (4 rows/partition)
            xt = []
            for c in range(NCH):
                t = big.tile([128, 512], fp32, tag=f"x{c}")
                src = xr[CHR * c : CHR * (c + 1), :].rearrange(
                    "(p a) d -> p (a d)", p=128
                )
                nc.sync.dma_start(t[:], src)
                xt.append(t)

            # ---- constants on GpSimd (no deps)
            ident = big.tile([128, 128], bf16, tag="ident")
            ones_sq = big.tile([128, 128], bf16, tag="ones_sq")
            nc.gpsimd.memset(ones_sq[:], 1.0)
            nc.gpsimd.affine_select(
                ident[:], ones_sq[:], [[1, 128]], ALU.is_equal, 0.0,
                base=0, channel_multiplier=-1,
            )
            # ones_cols[:, 16c+c] = 1 -> lhsT slice c routes a colsum into
            # partition c of the [16, N] PSUM accumulator
            ones_cols = big.tile([128, 256], bf16, tag="ones_cols")
            nc.gpsimd.memset(ones_cols[:], 0.0)
            for c in range(1, 16):
                nc.gpsimd.memset(ones_cols[:, 17 * c : 17 * c + 1], 1.0)

            # dummy exp pins the exp table set before any Square/Copy runs
            dzero = big.tile([128, 1], fp32, tag="dzero")
            nc.gpsimd.memset(dzero[:], 0.0)
            dout = big.tile([128, 1], fp32, tag="dout")
            nc.scalar.activation(dout[:], dzero[:], AF.Exp)

            # ---- squares for chunks 0..8 on ACT (idle pre-stream window)
            sq_tiles = {}
            for c in range(9):
                sqt = sqp.tile([128, 512], fp32, tag="sq")
                nc.scalar.activation(sqt[:], xt[c][:], AF.Square)
                sq_tiles[c] = sqt

            def sq_gps(lo, hi):
                for c in range(lo, hi):
                    sqt = sqp.tile([128, 512], fp32, tag="sq")
                    nc.gpsimd.tensor_mul(sqt[:], xt[c][:], xt[c][:])
                    sq_tiles[c] = sqt

            # ---- per-chunk row-norms² (DVE reduce) + rsqrt via bit trick
            inv_batches = [(0, 4), (4, 9), (9, 13), (13, 16)]
            invt = {}

            def rsqrt_batch(bi):
                lo, hi = inv_batches[bi]
                w = 4 * (hi - lo)
                n2 = big.tile([128, w], fp32, tag=f"n2_{bi}")
                for c in range(lo, hi):
                    nc.vector.tensor_reduce(
                        n2[:, 4 * (c - lo) : 4 * (c - lo) + 4],
                        sq_tiles[c][:].rearrange("p (a d) -> p a d", d=128),
                        axis=AX.X, op=ALU.add,
                    )
                t0 = big.tile([128, w], i32, tag=f"rs0_{bi}")
                nc.vector.tensor_scalar(
                    t0[:], n2[:].bitcast(i32), 1, None, op0=ALU.logical_shift_right
                )
                t1 = big.tile([128, w], i32, tag=f"rs1_{bi}")
                nc.vector.tensor_scalar(t1[:], t0[:], MAGIC + 1, None, op0=ALU.subtract)
                seed = big.tile([128, w], i32, tag=f"rs2_{bi}")
                nc.vector.tensor_scalar(seed[:], t1[:], -1, None, op0=ALU.bitwise_xor)
                h = big.tile([128, w], fp32, tag=f"rsh_{bi}")
                nc.vector.tensor_scalar(h[:], n2[:], 0.5, None, op0=ALU.mult)
                cur = seed[:].bitcast(fp32)
                for it in range(2):
                    t = big.tile([128, w], fp32, tag=f"rst_{bi}_{it}")
                    nc.vector.tensor_mul(t[:], cur, cur)
                    u = big.tile([128, w], fp32, tag=f"rsu_{bi}_{it}")
                    nc.vector.tensor_mul(u[:], h[:], t[:])
                    wt = big.tile([128, w], fp32, tag=f"rsw_{bi}_{it}")
                    nc.vector.tensor_scalar(
                        wt[:], u[:], 1.5, -1.0, op0=ALU.subtract, op1=ALU.mult
                    )
                    r2 = big.tile([128, w], fp32, tag=f"rsr_{bi}_{it}")
                    nc.vector.tensor_mul(r2[:], cur, wt[:])
                    cur = r2[:]
                invt[bi] = cur

            def inv_col(c):
                for bi, (lo, hi) in enumerate(inv_batches):
                    if lo <= c < hi:
                        return invt[bi][:, 4 * (c - lo) : 4 * (c - lo) + 4]
                raise AssertionError

            # ---- scales (z = x * inv, bf16): subs 0,1 on GpSimd, 2,3 on DVE
            zt = {}

            def scale_chunk(c):
                z = big.tile([128, 512], bf16, tag=f"z{c}")
                ic = inv_col(c)
                for j in range(4):
                    eng = nc.gpsimd if j < 2 else nc.vector
                    eng.tensor_scalar(
                        z[:, 128 * j : 128 * (j + 1)],
                        xt[c][:, 128 * j : 128 * (j + 1)],
                        ic[:, j : j + 1], None, op0=ALU.mult,
                    )
                zt[c] = z

            # ---- transposes: PE matmul with identity into PSUM, then copy
            # to SBUF (ACT for chunks <=2 = pre-stream, DVE after)
            zTt = {}

            RS = big.tile([128, 24], fp32, tag="RS")

            with (
                tc.tile_pool(name="ptp", bufs=1, space=bass.MemorySpace.PSUM) as ptp,
                tc.tile_pool(name="psp", bufs=2, space=bass.MemorySpace.PSUM) as psp,
                tc.tile_pool(name="cap", bufs=1, space=bass.MemorySpace.PSUM) as cap,
            ):

                def transpose_chunk(c):
                    pt = ptp.tile([128, 512], fp32, tag="pt")
                    for j in range(4):
                        nc.tensor.matmul(
                            pt[:, 128 * j : 128 * (j + 1)],
                            zt[c][:, 128 * j : 128 * (j + 1)],
                            ident[:], start=True, stop=True,
                        )
                    zT = big.tile([128, 512], bf16, tag=f"zT{c}")
                    if c <= 2:
                        nc.scalar.activation(zT[:], pt[:], AF.Copy)
                    else:
                        nc.vector.tensor_copy(zT[:], pt[:])
                    zTt[c] = zT

                colacc = cap.tile([16, 512], fp32, tag="colacc")
                n_colmm = 4 * sum(len(chs) - 1 for _, chs in GROUPS)
                col_state = {"i": 0}
                je_all = {}

                def group_main(gi):
                    lhsc, chs = GROUPS[gi]
                    W = 512 * len(chs)
                    for sub in range(4):
                        lhsT = zTt[lhsc][:, 128 * sub : 128 * (sub + 1)]
                        ps = psp.tile([128, 1536], fp32, tag="ps")
                        for i, c in enumerate(chs):
                            nc.tensor.matmul(
                                ps[:, 512 * i : 512 * (i + 1)], lhsT, zTt[c][:],
                                start=True, stop=True,
                            )
                        je = jep.tile([128, 1536], bf16, tag="je")
                        idx = 4 * gi + sub
                        nc.scalar.activation(
                            je[:, :W], ps[:, :W], AF.Exp, scale=2.0,
                            accum_out=RS[:, idx : idx + 1],
                        )
                        je_all[(gi, sub)] = je

                def group_col(gi):
                    lhsc, chs = GROUPS[gi]
                    for i, c in enumerate(chs):
                        if c == lhsc:
                            continue
                        for sub in range(4):
                            ci = col_state["i"]
                            nc.tensor.matmul(
                                colacc[:],
                                ones_cols[:, 16 * c : 16 * (c + 1)],
                                je_all[(gi, sub)][:, 512 * i : 512 * (i + 1)],
                                start=(ci == 0),
                                stop=(ci == n_colmm - 1),
                            )
                            col_state["i"] = ci + 1

                # ---- hand-scheduled emission (per-engine queues in order):
                rsqrt_batch(0)
                for c in range(0, 4):
                    scale_chunk(c)
                    transpose_chunk(c)
                sq_gps(9, 13)
                rsqrt_batch(1)
                group_main(0)
                for c in range(4, 9):
                    scale_chunk(c)
                    transpose_chunk(c)
                sq_gps(13, 16)
                rsqrt_batch(2)
                group_main(1)
                group_col(0)
                for c in range(9, 13):
                    scale_chunk(c)
                group_main(2)
                group_col(1)
                for c in range(9, 12):
                    transpose_chunk(c)
                rsqrt_batch(3)
                group_main(3)
                group_col(2)
                transpose_chunk(12)
                for c in range(13, 16):
                    scale_chunk(c)
                transpose_chunk(13)
                group_main(4)
                group_col(3)
                transpose_chunk(14)
                transpose_chunk(15)
                group_main(5)
                group_col(4)
                group_col(5)

                # ---- tail: per-subgroup row partials, positives, colacc out
                rowp = big.tile([128, 12], fp32, tag="rowp")
                nc.vector.tensor_reduce(
                    rowp[:, 0:4],
                    RS[:, 0:12].rearrange("p (g s) -> p s g", g=3),
                    axis=AX.X, op=ALU.add,
                )
                nc.vector.tensor_reduce(
                    rowp[:, 4:8],
                    RS[:, 12:24].rearrange("p (g s) -> p s g", g=3),
                    axis=AX.X, op=ALU.add,
                )
                pm = big.tile([128, 512], fp32, tag="pm")
                nc.vector.tensor_mul(pm[:], zt[0][:], zt[8][:])
                nc.vector.tensor_reduce(
                    rowp[:, 8:12],
                    pm[:].rearrange("p (a d) -> p a d", d=128),
                    axis=AX.X, op=ALU.add,
                )
                cs = big.tile([16, 512], fp32, tag="cs")
                nc.vector.tensor_copy(cs[:], colacc[:])
                nc.sync.dma_start(rowp_out[:], rowp[:])
                nc.sync.dma_start(colp_out[:], cs[:])

    nc.compile()
    return nc


def _get_nc():
    if "nc" not in _CACHE:
        _CACHE["nc"] = _build_nc()
    return _CACHE["nc"]


def _in_maps(proj_1, proj_2):
    p1 = np.asarray(proj_1, dtype=np.float32)
    p2 = np.asarray(proj_2, dtype=np.float32)
    X = np.concatenate([p1, p2], axis=0)
    maps = []
    for k in range(NCORES):
        order = [(k + c) % NCH for c in range(NCH)]
        xrot = np.ascontiguousarray(
            np.concatenate([X[CHR * g : CHR * (g + 1)] for g in order], axis=0)
        )
        maps.append({"xrot": xrot})
    return maps


def _assemble(results):
    # x/z tiles hold 4 rows per partition: tile free col 128j+d <-> chunk
    # row 4p+j, so transposed zT free index r = 128j+p <-> chunk row 4p+j.
    rowsum = np.zeros(2 * BATCH, dtype=np.float64)
    possum = 0.0
    p_idx = np.arange(128)
    r_idx = np.arange(512)
    for k in range(NCORES):
        rp = results[k]["rowp"].astype(np.float64)
        cp = results[k]["colp"].astype(np.float64)
        for sidx, lc in ((0, 0), (4, 8)):
            base = CHR * ((k + lc) % NCH)
            for s in range(4):
                rowsum[base + 4 * p_idx + s] += rp[:, sidx + s]
        for c in range(1, 16):
            base = CHR * ((k + c) % NCH)
            rowsum[base + 4 * (r_idx % 128) + r_idx // 128] += cp[c]
        possum += float(rp[:, 8:12].sum())
    denom = rowsum - E2
    loss = (np.log(denom).sum() - 4.0 * possum) / (2 * BATCH)
    return np.float32(loss)


def _run(proj_1, proj_2, trace=False):
    from concourse.bass_utils import run_bass_kernel_spmd

    nc = _get_nc()
    res = run_bass_kernel_spmd(
        nc, _in_maps(proj_1, proj_2), list(range(NCORES)), trace=trace
    )
    loss = _assemble(res.results)
    return loss, res


def kernel(proj_1, proj_2):
    loss, _ = _run(proj_1, proj_2, trace=False)
    return loss


# revision 10
# speedup vs baseline: 1.6292x; 1.5939x over previous
"""NT-Xent contrastive loss on 8 Trainium2 NeuronCores — v2.

Exploits sim-matrix symmetry to HALVE the exp work vs v1 (the ACT
engine's exp stream is the critical path): the 8192x8192 sim matrix is
cut into a 16x16 grid of 512-row chunks; each unordered chunk pair is
computed ONCE. Core k owns block-rows k and k+8 (circulant): row k
sweeps columns k..k+8, row k+8 sweeps k+8..k+15 (mod 16) — 17 blocks
per core (9+8), exactly 1/8 of the 136 upper-triangle blocks.
Row-sums of each exp block come free via ACT accumulate; the mirrored
row-sums (transposed blocks) come from column-sums computed on the PE
with ones-column lhsT matmuls into a persistent [16,512] PSUM
accumulator. Host assembles rowsums, subtracts the diagonal e^2, and
does the final ln/sum in float64 (off the HW critical path).

Inputs are chunk-ROTATED per core on the host (local chunk c = global
chunk (k+c) mod 16), so the device program is identical across cores.

Head avoids ACT-table thrash entirely: 1/sqrt(n2) is a quake-style
bit-trick rsqrt on DVE (shift/xor on bitcast int32 + 2 Newton steps),
so ACT only ever runs Square/Copy/Exp — all present in the exp table
set (a dummy exp up front pins it; its single table load lands in the
idle boot window). Squares for chunks 0..8 run on ACT (idle until the
exp stream starts), 9..15 on GpSimd; scales split GpSimd/DVE;
PSUM->SBUF transpose copies go ACT (first 3, pre-stream) / DVE (rest).
Emission order is hand-scheduled so no in-order engine queue blocks
the ACT exp stream on a late-chunk dependency.
"""

import sys

sys.path.insert(0, "/opt/trn_rl_repo")

import numpy as np

BATCH = 4096
DIM = 128
NCORES = 8
NCH = 16  # 512-row chunks of concat(z1, z2)
CHR = 512
E2 = float(np.exp(2.0))
MAGIC = 0x5F3759DF

# (lhs chunk, rhs chunks) per PSUM group; RS col = 4*gi + sub
GROUPS = [
    (0, [0, 1, 2]),
    (0, [3, 4, 5]),
    (0, [6, 7, 8]),
    (8, [8, 9, 10]),
    (8, [11, 12, 13]),
    (8, [14, 15]),
]

_CACHE = {}


def _build_nc():
    import concourse.bacc as bacc
    import concourse.bass as bass
    import concourse.mybir as mybir
    import concourse.tile as tile

    fp32 = mybir.dt.float32
    bf16 = mybir.dt.bfloat16
    i32 = mybir.dt.int32
    AF = mybir.ActivationFunctionType
    ALU = mybir.AluOpType
    AX = mybir.AxisListType

    nc = bacc.Bacc("TRN2", target_bir_lowering=False, debug=False, num_devices=NCORES)
    xr = nc.declare_dram_parameter("xrot", [2 * BATCH, DIM], fp32, isOutput=False)
    rowp_out = nc.declare_dram_parameter("rowp", [128, 12], fp32, isOutput=True)
    colp_out = nc.declare_dram_parameter("colp", [16, 512], fp32, isOutput=True)

    with tile.TileContext(nc) as tc:
        with (
            tc.tile_pool(name="big", bufs=1) as big,
            tc.tile_pool(name="sqp", bufs=6) as sqp,
            tc.tile_pool(name="jep", bufs=9) as jep,
        ):
            # ---- input loads: 16 chunk tiles [128, 512] (4 rows/partition)
            xt = []
            for c in range(NCH):
                t = big.tile([128, 512], fp32, tag=f"x{c}")
                src = xr[CHR * c : CHR * (c + 1), :].rearrange(
                    "(p a) d -> p (a d)", p=128
                )
                nc.sync.dma_start(t[:], src)
                xt.append(t)

            # ---- constants on GpSimd (no deps)
            ident = big.tile([128, 128], bf16, tag="ident")
            ones_sq = big.tile([128, 128], bf16, tag="ones_sq")
            nc.gpsimd.memset(ones_sq[:], 1.0)
            nc.gpsimd.affine_select(
                ident[:], ones_sq[:], [[1, 128]], ALU.is_equal, 0.0,
                base=0, channel_multiplier=-1,
            )
            # ones_cols[:, 16c+c] = 1 -> lhsT slice c routes a colsum into
            # partition c of the [16, N] PSUM accumulator
            ones_cols = big.tile([128, 256], bf16, tag="ones_cols")
            nc.gpsimd.memset(ones_cols[:], 0.0)
            for c in range(1, 16):
                nc.gpsimd.memset(ones_cols[:, 17 * c : 17 * c + 1], 1.0)

            # dummy exp pins the exp table set before any Square/Copy runs
            dzero = big.tile([128, 1], fp32, tag="dzero")
            nc.gpsimd.memset(dzero[:], 0.0)
            dout = big.tile([128, 1], fp32, tag="dout")
            nc.scalar.activation(dout[:], dzero[:], AF.Exp)

            # ---- per-chunk row-norms²: squares on ACT for chunks 0..8
            # (idle pre-stream window) / DVE for 9..15; DVE reduce; rsqrt
            # via bit trick + 1 Newton step (rel err ~1e-3, way inside
            # tolerance). GpSimd is banned from the data pipeline (its Q7
            # ops cost ~2µs fixed AND stall concurrent DVE ops; measured).
            sq_tiles = {}

            def sq_act(lo, hi):
                for c in range(lo, hi):
                    sqt = sqp.tile([128, 512], fp32, tag="sq")
                    nc.scalar.activation(sqt[:], xt[c][:], AF.Square)
                    sq_tiles[c] = sqt

            def sq_dve(lo, hi):
                for c in range(lo, hi):
                    sqt = sqp.tile([128, 512], fp32, tag="sq")
                    nc.vector.tensor_mul(sqt[:], xt[c][:], xt[c][:])
                    sq_tiles[c] = sqt

            inv_batches = [(0, 4), (4, 9), (9, 14), (14, 16)]
            invt = {}

            def rsqrt_batch(bi):
                lo, hi = inv_batches[bi]
                w = 4 * (hi - lo)
                n2 = big.tile([128, w], fp32, tag=f"n2_{bi}")
                for c in range(lo, hi):
                    nc.vector.tensor_reduce(
                        n2[:, 4 * (c - lo) : 4 * (c - lo) + 4],
                        sq_tiles[c][:].rearrange("p (a d) -> p a d", d=128),
                        axis=AX.X, op=ALU.add,
                    )
                t0 = big.tile([128, w], i32, tag=f"rs0_{bi}")
                nc.vector.tensor_scalar(
                    t0[:], n2[:].bitcast(i32), 1, None, op0=ALU.logical_shift_right
                )
                t1 = big.tile([128, w], i32, tag=f"rs1_{bi}")
                nc.vector.tensor_scalar(t1[:], t0[:], MAGIC + 1, None, op0=ALU.subtract)
                seed = big.tile([128, w], i32, tag=f"rs2_{bi}")
                nc.vector.tensor_scalar(seed[:], t1[:], -1, None, op0=ALU.bitwise_xor)
                cur = seed[:].bitcast(fp32)
                # one Newton step r·(1.5 − 0.5·v·r²) = r·((v·r² − 3)·(−0.5))
                t = big.tile([128, w], fp32, tag=f"rst_{bi}")
                nc.vector.tensor_mul(t[:], cur, cur)
                u = big.tile([128, w], fp32, tag=f"rsu_{bi}")
                nc.vector.tensor_mul(u[:], n2[:], t[:])
                wt = big.tile([128, w], fp32, tag=f"rsw_{bi}")
                nc.vector.tensor_scalar(
                    wt[:], u[:], 3.0, -0.5, op0=ALU.subtract, op1=ALU.mult
                )
                r2 = big.tile([128, w], fp32, tag=f"rsr_{bi}")
                nc.vector.tensor_mul(r2[:], cur, wt[:])
                invt[bi] = r2[:]

            def inv_col(c):
                for bi, (lo, hi) in enumerate(inv_batches):
                    if lo <= c < hi:
                        return invt[bi][:, 4 * (c - lo) : 4 * (c - lo) + 4]
                raise AssertionError

            # ---- scales (z = x * inv, bf16): chunks 0..2 ride ACT's idle
            # pre-stream window (Copy with per-partition scale AP), rest DVE
            zt = {}

            def scale_chunk(c):
                z = big.tile([128, 512], bf16, tag=f"z{c}")
                ic = inv_col(c)
                for j in range(4):
                    if c <= 2:
                        nc.scalar.activation(
                            z[:, 128 * j : 128 * (j + 1)],
                            xt[c][:, 128 * j : 128 * (j + 1)],
                            AF.Copy, scale=ic[:, j : j + 1],
                        )
                    else:
                        nc.vector.tensor_scalar(
                            z[:, 128 * j : 128 * (j + 1)],
                            xt[c][:, 128 * j : 128 * (j + 1)],
                            ic[:, j : j + 1], None, op0=ALU.mult,
                        )
                zt[c] = z

            # ---- transposes: PE matmul with identity into PSUM, then copy
            # to SBUF (ACT for chunks <=2 = pre-stream, DVE after)
            zTt = {}

            RS = big.tile([128, 24], fp32, tag="RS")

            with (
                tc.tile_pool(name="ptp", bufs=1, space=bass.MemorySpace.PSUM) as ptp,
                tc.tile_pool(name="psp", bufs=2, space=bass.MemorySpace.PSUM) as psp,
                tc.tile_pool(name="cap", bufs=1, space=bass.MemorySpace.PSUM) as cap,
            ):

                def transpose_chunk(c):
                    pt = ptp.tile([128, 512], fp32, tag="pt")
                    for j in range(4):
                        nc.tensor.matmul(
                            pt[:, 128 * j : 128 * (j + 1)],
                            zt[c][:, 128 * j : 128 * (j + 1)],
                            ident[:], start=True, stop=True,
                        )
                    zT = big.tile([128, 512], bf16, tag=f"zT{c}")
                    if c <= 2:
                        nc.scalar.activation(zT[:], pt[:], AF.Copy)
                    else:
                        nc.vector.tensor_copy(zT[:], pt[:])
                    zTt[c] = zT

                colacc = cap.tile([16, 512], fp32, tag="colacc")
                n_colmm = 4 * sum(len(chs) - 1 for _, chs in GROUPS)
                col_state = {"i": 0}
                je_all = {}

                def group_main(gi):
                    lhsc, chs = GROUPS[gi]
                    W = 512 * len(chs)
                    for sub in range(4):
                        lhsT = zTt[lhsc][:, 128 * sub : 128 * (sub + 1)]
                        ps = psp.tile([128, 1536], fp32, tag="ps")
                        for i, c in enumerate(chs):
                            nc.tensor.matmul(
                                ps[:, 512 * i : 512 * (i + 1)], lhsT, zTt[c][:],
                                start=True, stop=True,
                            )
                        je = jep.tile([128, 1536], bf16, tag="je")
                        idx = 4 * gi + sub
                        nc.scalar.activation(
                            je[:, :W], ps[:, :W], AF.Exp, scale=2.0,
                            accum_out=RS[:, idx : idx + 1],
                        )
                        je_all[(gi, sub)] = je

                def group_col(gi):
                    lhsc, chs = GROUPS[gi]
                    for i, c in enumerate(chs):
                        if c == lhsc:
                            continue
                        for sub in range(4):
                            ci = col_state["i"]
                            nc.tensor.matmul(
                                colacc[:],
                                ones_cols[:, 16 * c : 16 * (c + 1)],
                                je_all[(gi, sub)][:, 512 * i : 512 * (i + 1)],
                                start=(ci == 0),
                                stop=(ci == n_colmm - 1),
                            )
                            col_state["i"] = ci + 1

                # ---- hand-scheduled emission (per-engine queues in order):
                sq_act(0, 9)
                rsqrt_batch(0)
                for c in range(0, 4):
                    scale_chunk(c)
                    transpose_chunk(c)
                rsqrt_batch(1)
                group_main(0)
                for c in range(4, 9):
                    scale_chunk(c)
                    transpose_chunk(c)
                sq_dve(9, 14)
                group_main(1)
                group_col(0)
                rsqrt_batch(2)
                for c in range(9, 11):
                    scale_chunk(c)
                    transpose_chunk(c)
                group_main(2)
                group_col(1)
                for c in range(11, 14):
                    scale_chunk(c)
                    transpose_chunk(c)
                sq_dve(14, 16)
                rsqrt_batch(3)
                group_main(3)
                group_col(2)
                for c in range(14, 16):
                    scale_chunk(c)
                    transpose_chunk(c)
                group_main(4)
                group_col(3)
                group_main(5)
                group_col(4)
                group_col(5)

                # ---- tail: per-subgroup row partials, positives, colacc out
                rowp = big.tile([128, 12], fp32, tag="rowp")
                nc.vector.tensor_reduce(
                    rowp[:, 0:4],
                    RS[:, 0:12].rearrange("p (g s) -> p s g", g=3),
                    axis=AX.X, op=ALU.add,
                )
                nc.vector.tensor_reduce(
                    rowp[:, 4:8],
                    RS[:, 12:24].rearrange("p (g s) -> p s g", g=3),
                    axis=AX.X, op=ALU.add,
                )
                pm = big.tile([128, 512], fp32, tag="pm")
                nc.vector.tensor_mul(pm[:], zt[0][:], zt[8][:])
                nc.vector.tensor_reduce(
                    rowp[:, 8:12],
                    pm[:].rearrange("p (a d) -> p a d", d=128),
                    axis=AX.X, op=ALU.add,
                )
                cs = big.tile([16, 512], fp32, tag="cs")
                nc.vector.tensor_copy(cs[:], colacc[:])
                nc.sync.dma_start(rowp_out[:], rowp[:])
                nc.sync.dma_start(colp_out[:], cs[:])

    nc.compile()
    return nc


def _get_nc():
    if "nc" not in _CACHE:
        _CACHE["nc"] = _build_nc()
    return _CACHE["nc"]


def _in_maps(proj_1, proj_2):
    p1 = np.asarray(proj_1, dtype=np.float32)
    p2 = np.asarray(proj_2, dtype=np.float32)
    X = np.concatenate([p1, p2], axis=0)
    maps = []
    for k in range(NCORES):
        order = [(k + c) % NCH for c in range(NCH)]
        xrot = np.ascontiguousarray(
            np.concatenate([X[CHR * g : CHR * (g + 1)] for g in order], axis=0)
        )
        maps.append({"xrot": xrot})
    return maps


def _assemble(results):
    # x/z tiles hold 4 rows per partition: tile free col 128j+d <-> chunk
    # row 4p+j, so transposed zT free index r = 128j+p <-> chunk row 4p+j.
    rowsum = np.zeros(2 * BATCH, dtype=np.float64)
    possum = 0.0
    p_idx = np.arange(128)
    r_idx = np.arange(512)
    for k in range(NCORES):
        rp = results[k]["rowp"].astype(np.float64)
        cp = results[k]["colp"].astype(np.float64)
        for sidx, lc in ((0, 0), (4, 8)):
            base = CHR * ((k + lc) % NCH)
            for s in range(4):
                rowsum[base + 4 * p_idx + s] += rp[:, sidx + s]
        for c in range(1, 16):
            base = CHR * ((k + c) % NCH)
            rowsum[base + 4 * (r_idx % 128) + r_idx // 128] += cp[c]
        possum += float(rp[:, 8:12].sum())
    denom = rowsum - E2
    loss = (np.log(denom).sum() - 4.0 * possum) / (2 * BATCH)
    return np.float32(loss)


def _run(proj_1, proj_2, trace=False):
    from concourse.bass_utils import run_bass_kernel_spmd

    nc = _get_nc()
    res = run_bass_kernel_spmd(
        nc, _in_maps(proj_1, proj_2), list(range(NCORES)), trace=trace
    )
    loss = _assemble(res.results)
    return loss, res


def kernel(proj_1, proj_2):
    loss, _ = _run(proj_1, proj_2, trace=False)
    return loss
